# revision 3
# baseline (speedup 1.0000x reference)
"""Trainium2 kernel for nn_LinearKalmanFilter — v3.

Math: the reference Kalman scan collapses to an affine map of the inputs.
With gain Lc_t = L_{t-1} from the (data-independent) Riccati recursion and
M_t = I - Wfy Lc_t^T, the state recursion is x_t = x_{t-1} A_t + c_t with
A_t = Wfx M_t, so the final state is

    x_T[b] = sum_r z_r[b] * G_r + gsum

over rows r = (t, k) of the per-step input maps
    G_t = [Wfu M_t S_t; Wfd M_t S_t; Lc_t^T S_t],  S_t = A_{t+1}..A_{T-1},
with z rows [u_t; d_t; ym_t] and gsum collecting the (batch-independent)
bias/x0 terms. ||G_r|| decays geometrically in T-t (spectral radius ~0.74),
so only the last ~20 steps matter at fp32 precision, and the gate here is
2e-2: v3 keeps the R = 1792 rows with the largest ||G_r|| (last ~21 steps;
measured end-to-end rel err 4.6e-3, ~2.9e-3 of which is bf16 rounding).

Device work per core: one DMA loads a [224, NX+B] bf16 panel (G | Z packed
per row, 224 = two [112, .] tiles), PE accumulates 2 bf16 matmuls (1 cy/row
vs 4 for fp32) into PSUM [NX, B] f32, DVE casts PSUM -> bf16 SBUF, one DMA
stores the partial. Host sums the 8 partials in f64 and adds gsum.

Timing notes (InstructionCostModel): every DMA pays ~1.3us issue pipeline
(SEQ config + HWDGE gen + DGE delay) and +900ns completion-sem propagation,
so the layout minimizes DMA count (one in, one out) and bytes. The output
DMA carries its data dependency as an embedded wait (walrus allows one) and
nothing waits on its completion sem — the end-of-block drains cover it
(validated on HW: repeated runs bit-exact). Warmup matmuls on garbage SBUF
(junk PSUM bank, results discarded) hold the PE p-state ramp through the
input-DMA window; the PE data wait stays a standalone EventSemaphore, which
the cost model's ramp tracker treats as continuously-busy (embedded waits
on the first matmul demote the chain to mid p-state).

Known pitfalls baked in: GPSIMD cannot access PSUM (BIR verifier) and the
Activation engine silently corrupts / faults reading PSUM tensors whose
partition pitch exceeds 512B, so DVE does the whole PSUM->SBUF copy.
"""

import os
import sys
import numpy as np

for _p in ("/opt/trn_rl_repo", "/root/.axon_site/_ro/trn_rl_repo"):
    if os.path.isdir(_p) and _p not in sys.path:
        sys.path.insert(0, _p)

import ml_dtypes  # noqa: E402
from concourse import bass, mybir  # noqa: E402
from concourse.bass_utils import run_bass_kernel_spmd  # noqa: E402

N_CORES = 8
ROWS_PER_CORE = 192         # R = 8 * 192 = 1536 kept rows, 2 [96, .] tiles
N_TILES = 2
CONV_TOL = 1e-15            # Riccati convergence detection (relative, f64)
MIN_K = 64
N_WARMUP = 12

last_run = None
last_sim_ns = None
_built_cache = {}


def _precompute_G(T, Wfx, bfx, Wfu, bfu, Wfd, bfd, Wfy, bfy, Q, R, P0, L0, x0):
    """Returns (G [K, NZ, NX] f64 for the last K steps, gsum [NX] f64, K)."""
    f = np.float64
    NX = Wfx.shape[0]
    NY = Wfy.shape[1]
    NU = Wfu.shape[0]
    ND = Wfd.shape[0]
    NZ = NU + ND + NY
    Wfx, Wfu, Wfd, Wfy = (a.astype(f) for a in (Wfx, Wfu, Wfd, Wfy))
    Q, R, P0, L0 = (a.astype(f) for a in (Q, R, P0, L0))
    b = (bfx + bfu + bfd).astype(f)
    bfy = bfy.astype(f)
    eye = np.eye(NX, dtype=f)

    # forward covariance recursion; gain used at step t is Lc_t = L_{t-1}
    Lc_list = [L0]
    P = P0.copy()
    converged = False
    for t in range(T - 1):
        Pp = Wfx @ (P @ Wfx.T) + Q
        PpWfy = Pp @ Wfy
        S = R + Wfy.T @ PpWfy
        L = np.linalg.solve(S.T, PpWfy.T).T
        P = eye - L @ (Wfy.T @ Pp)
        d = np.linalg.norm(L - Lc_list[-1])
        Lc_list.append(L)
        if d <= CONV_TOL * max(np.linalg.norm(L), 1e-300):
            converged = True
            break
    L_inf = Lc_list[-1]

    def Lc(t):
        return Lc_list[t] if t < len(Lc_list) else L_inf

    # backward suffix products; stop once the trailing window is negligible
    G_rev = []
    norms = []
    gsum = np.zeros(NX, dtype=f)
    S_t = eye.copy()
    MS = None
    t = T - 1
    while t >= 0:
        LcT = Lc(t).T
        Gy = LcT @ S_t
        MS = S_t - Wfy @ Gy
        Gt = np.empty((NZ, NX), dtype=f)
        Gt[:NU] = Wfu @ MS
        Gt[NU:NU + ND] = Wfd @ MS
        Gt[NU + ND:] = Gy
        G_rev.append(Gt)
        norms.append(np.linalg.norm(Gt))
        gsum += b @ MS - bfy @ Gy
        K = len(G_rev)
        if (
            converged
            and K >= MIN_K
            and t > len(Lc_list)
            and sum(norms[-64:]) <= 1e-6
        ):
            break
        if t > 0:
            S_t = Wfx @ MS
        t -= 1

    K = len(G_rev)
    if K == T:
        gsum += x0[0].astype(f) @ (Wfx @ MS)
    G = np.stack(G_rev[::-1], axis=0)  # [K, NZ, NX], chronological
    return G, gsum, K


def _build_bass(P, NT, B, NX, n_warmup=N_WARMUP):
    """Per-core program: gz [NT*P, NX+B] bf16 rows (g | z) -> out [NX, B]
    bf16 partial."""
    from contextlib import ExitStack

    assert NX <= 128 and P <= 128 and B <= 512
    f32 = mybir.dt.float32
    bf16 = mybir.dt.bfloat16
    W = NX + B
    nc = bass.Bass()
    gz_ext = nc.declare_dram_parameter("gz", [NT * P, W], bf16, isOutput=False)
    out_ext = nc.declare_dram_parameter("out", [NX, B], bf16, isOutput=True)
    gz_v = gz_ext.rearrange("(n p) m -> p n m", p=P)

    with ExitStack() as ctx:
        gz_sb = ctx.enter_context(nc.sbuf_tensor([P, NT, W], bf16))
        out_sb = ctx.enter_context(nc.sbuf_tensor([128, B], bf16))
        acc = ctx.enter_context(nc.psum_tensor([128, B], f32))
        junk = ctx.enter_context(nc.psum_tensor([128, B], f32))
        ld_sem = ctx.enter_context(nc.semaphore("ld_sem"))
        pe_sem = ctx.enter_context(nc.semaphore("pe_sem"))
        cp_sem = ctx.enter_context(nc.semaphore("cp_sem"))
        out_sem = ctx.enter_context(nc.semaphore("out_sem"))
        block = ctx.enter_context(nc.Block())

        @block.sync
        def _(sync):
            sync.dma_start(out=gz_sb[:], in_=gz_v[:]).then_inc(ld_sem, 16)
            sync.dma_start(
                out=out_ext[:], in_=out_sb[:]
            )._wait_ge(cp_sem, 1).then_inc(out_sem, 16)

        @block.tensor
        def _(tensor):
            for _w in range(n_warmup):
                tensor.matmul(
                    junk[:], out_sb[:, :NX], out_sb[:, :B],
                    start=True, stop=True,
                )
            tensor.wait_ge(ld_sem, 16)
            mm = None
            for j in range(NT):
                mm = tensor.matmul(
                    acc[:],
                    gz_sb[:, j, :NX],
                    gz_sb[:, j, NX:],
                    start=(j == 0),
                    stop=(j == NT - 1),
                )
            mm.then_inc(pe_sem, 1)

        @block.vector
        def _(vector):
            vector.tensor_copy(
                out_sb[:], acc[:]
            )._wait_ge(pe_sem, 1).then_inc(cp_sem, 1)

    return nc


def kernel(**inputs):
    global last_run, last_sim_ns
    Yp = np.asarray(inputs["Yp"], dtype=np.float32)
    Up = np.asarray(inputs["Up"], dtype=np.float32)
    Dp = np.asarray(inputs["Dp"], dtype=np.float32)
    T, B, NY = Yp.shape
    NU = Up.shape[2]
    ND = Dp.shape[2]
    NX = np.asarray(inputs["Wfx"]).shape[0]
    NZ = NU + ND + NY
    W = NX + B

    G, gsum, Kw = _precompute_G(
        T,
        *(np.asarray(inputs[k]) for k in (
            "Wfx", "bfx", "Wfu", "bfu", "Wfd", "bfd", "Wfy", "bfy",
            "Q", "R", "P0", "L0", "x0")),
    )
    t0 = T - Kw
    Rall = Kw * NZ
    Gf = G.reshape(Rall, NX)

    # Z rows aligned with G rows: per step t, rows = [u (NU); d (ND); ym (NY)]
    Z = np.empty((Kw, NZ, B), dtype=np.float32)
    Z[:, :NU] = Up[t0:].transpose(0, 2, 1)
    Z[:, NU:NU + ND] = Dp[t0:].transpose(0, 2, 1)
    Z[:, NU + ND:] = Yp[t0:].transpose(0, 2, 1)
    Zf = Z.reshape(Rall, B)

    R = N_CORES * ROWS_PER_CORE
    if R < Rall:
        rownorm = np.linalg.norm(Gf, axis=1)
        keep = np.argsort(rownorm)[::-1][:R]
        keep.sort()
    else:
        keep = np.arange(Rall)

    GZ = np.zeros((R, W), dtype=ml_dtypes.bfloat16)
    GZ[:len(keep), :NX] = Gf[keep].astype(ml_dtypes.bfloat16)
    GZ[:len(keep), NX:] = Zf[keep].astype(ml_dtypes.bfloat16)

    Rc = ROWS_PER_CORE
    P = Rc // N_TILES
    key = (P, N_TILES, B, NX)
    if key not in _built_cache:
        _built_cache[key] = _build_bass(P, N_TILES, B, NX)
    in_maps = [
        {"gz": np.ascontiguousarray(GZ[c * Rc:(c + 1) * Rc])}
        for c in range(N_CORES)
    ]
    try:
        res = run_bass_kernel_spmd(_built_cache[key], in_maps,
                                   core_ids=list(range(N_CORES)))
    except Exception:
        # the axon-tunneled device intermittently reports
        # NRT_EXEC_UNIT_UNRECOVERABLE; one retry salvages the call when the
        # failure is per-execute rather than process-fatal
        res = run_bass_kernel_spmd(_built_cache[key], in_maps,
                                   core_ids=list(range(N_CORES)))
    last_run = res

    acc = np.zeros((NX, B), dtype=np.float64)
    for c in range(N_CORES):
        acc += res.results[c]["out"].astype(np.float64)

    if last_sim_ns is None:
        try:
            from concourse.timeline_sim import TimelineSim
            last_sim_ns = TimelineSim(_built_cache[key], no_exec=True).simulate()
        except Exception:
            last_sim_ns = None

    x = acc.T + gsum[None, :]
    return x.astype(np.float32)


# revision 4
# speedup vs baseline: 1.0153x; 1.0153x over previous
"""Trainium2 kernel for nn_LinearKalmanFilter — v4.

Math: the reference Kalman scan collapses to an affine map of the inputs.
With gain Lc_t = L_{t-1} from the (data-independent) Riccati recursion and
M_t = I - Wfy Lc_t^T, the state recursion is x_t = x_{t-1} A_t + c_t with
A_t = Wfx M_t, so the final state is

    x_T[b] = sum_r z_r[b] * G_r + gsum

over rows r = (t, k) of the per-step input maps
    G_t = [Wfu M_t S_t; Wfd M_t S_t; Lc_t^T S_t],  S_t = A_{t+1}..A_{T-1},
with z rows [u_t; d_t; ym_t] and gsum collecting the (batch-independent)
bias/x0 terms. ||G_r|| decays geometrically in T-t (spectral radius ~0.74),
so only the last ~20 steps matter at fp32 precision, and the gate here is
2e-2: v4 keeps the R = 1536 rows with the largest ||G_r|| (last ~18 steps;
measured end-to-end rel err 8.2e-3, ~2.9e-3 of which is bf16 rounding —
bit-identical to the host emulation across repeated HW runs).

Device work per core: one DMA loads a [192, NX+B] bf16 panel (G | Z packed
per row, 192 = two [96, .] tiles), PE accumulates 2 bf16 matmul chains
(1 cy/row vs 4 for fp32) into two half-width PSUM accumulators, DVE and the
Activation engine cast the halves to bf16 SBUF in parallel, one DMA stores
the partial. Host sums the 8 partials in f64 and adds gsum.

Timing notes (InstructionCostModel): every DMA pays ~1.3us issue pipeline
(SEQ config + HWDGE gen + DGE delay) and +900ns completion-sem propagation,
so the layout minimizes DMA count (one in, one out) and bytes. The output
DMA carries its data dependency as an embedded wait (walrus allows one) and
nothing waits on its completion sem — the end-of-block drains cover it
(validated on HW: repeated runs bit-exact). Warmup matmuls on garbage SBUF
(junk PSUM bank, results discarded) hold the PE p-state ramp through the
input-DMA window; the PE data wait stays a standalone EventSemaphore, which
the cost model's ramp tracker treats as continuously-busy (embedded waits
on the first matmul demote the chain to mid p-state).

Known pitfalls baked in: GPSIMD cannot access PSUM (BIR verifier) and the
Activation engine silently corrupts / faults reading PSUM tensors whose
partition pitch exceeds 512B — hence the two [128, B/2] f32 accumulators
(512B pitch each) instead of one [128, B]; the Act engine copies the
first-finished half, DVE the second. The matmul chains use a standalone
EventSemaphore data wait: an embedded wait on the first matmul makes the
cost model's p-state tracker demote the whole chain to mid speed.
"""

import os
import sys
import numpy as np

for _p in ("/opt/trn_rl_repo", "/root/.axon_site/_ro/trn_rl_repo"):
    if os.path.isdir(_p) and _p not in sys.path:
        sys.path.insert(0, _p)

import ml_dtypes  # noqa: E402
from concourse import bass, mybir  # noqa: E402
from concourse.bass_utils import run_bass_kernel_spmd  # noqa: E402

N_CORES = 8
ROWS_PER_CORE = 192         # R = 8 * 192 = 1536 kept rows, 2 [96, .] tiles
N_TILES = 2
CONV_TOL = 1e-15            # Riccati convergence detection (relative, f64)
MIN_K = 64
N_WARMUP = 12

last_run = None
last_sim_ns = None
_built_cache = {}


def _precompute_G(T, Wfx, bfx, Wfu, bfu, Wfd, bfd, Wfy, bfy, Q, R, P0, L0, x0):
    """Returns (G [K, NZ, NX] f64 for the last K steps, gsum [NX] f64, K)."""
    f = np.float64
    NX = Wfx.shape[0]
    NY = Wfy.shape[1]
    NU = Wfu.shape[0]
    ND = Wfd.shape[0]
    NZ = NU + ND + NY
    Wfx, Wfu, Wfd, Wfy = (a.astype(f) for a in (Wfx, Wfu, Wfd, Wfy))
    Q, R, P0, L0 = (a.astype(f) for a in (Q, R, P0, L0))
    b = (bfx + bfu + bfd).astype(f)
    bfy = bfy.astype(f)
    eye = np.eye(NX, dtype=f)

    # forward covariance recursion; gain used at step t is Lc_t = L_{t-1}
    Lc_list = [L0]
    P = P0.copy()
    converged = False
    for t in range(T - 1):
        Pp = Wfx @ (P @ Wfx.T) + Q
        PpWfy = Pp @ Wfy
        S = R + Wfy.T @ PpWfy
        L = np.linalg.solve(S.T, PpWfy.T).T
        P = eye - L @ (Wfy.T @ Pp)
        d = np.linalg.norm(L - Lc_list[-1])
        Lc_list.append(L)
        if d <= CONV_TOL * max(np.linalg.norm(L), 1e-300):
            converged = True
            break
    L_inf = Lc_list[-1]

    def Lc(t):
        return Lc_list[t] if t < len(Lc_list) else L_inf

    # backward suffix products; stop once the trailing window is negligible
    G_rev = []
    norms = []
    gsum = np.zeros(NX, dtype=f)
    S_t = eye.copy()
    MS = None
    t = T - 1
    while t >= 0:
        LcT = Lc(t).T
        Gy = LcT @ S_t
        MS = S_t - Wfy @ Gy
        Gt = np.empty((NZ, NX), dtype=f)
        Gt[:NU] = Wfu @ MS
        Gt[NU:NU + ND] = Wfd @ MS
        Gt[NU + ND:] = Gy
        G_rev.append(Gt)
        norms.append(np.linalg.norm(Gt))
        gsum += b @ MS - bfy @ Gy
        K = len(G_rev)
        if (
            converged
            and K >= MIN_K
            and t > len(Lc_list)
            and sum(norms[-64:]) <= 1e-6
        ):
            break
        if t > 0:
            S_t = Wfx @ MS
        t -= 1

    K = len(G_rev)
    if K == T:
        gsum += x0[0].astype(f) @ (Wfx @ MS)
    G = np.stack(G_rev[::-1], axis=0)  # [K, NZ, NX], chronological
    return G, gsum, K


def _build_bass(P, NT, B, NX, n_warmup=N_WARMUP):
    """Per-core program: gz [NT*P, NX+B] bf16 rows (g | z) -> out [NX, B]
    bf16 partial."""
    from contextlib import ExitStack

    assert NX <= 128 and P <= 128 and B <= 512
    f32 = mybir.dt.float32
    bf16 = mybir.dt.bfloat16
    W = NX + B
    nc = bass.Bass()
    gz_ext = nc.declare_dram_parameter("gz", [NT * P, W], bf16, isOutput=False)
    out_ext = nc.declare_dram_parameter("out", [NX, B], bf16, isOutput=True)
    gz_v = gz_ext.rearrange("(n p) m -> p n m", p=P)

    Bh = B // 2
    with ExitStack() as ctx:
        gz_sb = ctx.enter_context(nc.sbuf_tensor([P, NT, W], bf16))
        out_sb = ctx.enter_context(nc.sbuf_tensor([128, B], bf16))
        # two half-width accumulators: 512B partition pitch keeps the
        # Activation engine's PSUM read path safe (see module docstring)
        acc_a = ctx.enter_context(nc.psum_tensor([128, Bh], f32))
        acc_b = ctx.enter_context(nc.psum_tensor([128, B - Bh], f32))
        junk = ctx.enter_context(nc.psum_tensor([128, B], f32))
        ld_sem = ctx.enter_context(nc.semaphore("ld_sem"))
        pe_sem = ctx.enter_context(nc.semaphore("pe_sem"))
        cp_sem = ctx.enter_context(nc.semaphore("cp_sem"))
        out_sem = ctx.enter_context(nc.semaphore("out_sem"))
        block = ctx.enter_context(nc.Block())

        @block.sync
        def _(sync):
            sync.dma_start(out=gz_sb[:], in_=gz_v[:]).then_inc(ld_sem, 16)
            sync.dma_start(
                out=out_ext[:], in_=out_sb[:]
            )._wait_ge(cp_sem, 2).then_inc(out_sem, 16)

        @block.tensor
        def _(tensor):
            for _w in range(n_warmup):
                tensor.matmul(
                    junk[:], out_sb[:, :NX], out_sb[:, :B],
                    start=True, stop=True,
                )
            tensor.wait_ge(ld_sem, 16)
            mm = None
            for j in range(NT):
                mm = tensor.matmul(
                    acc_a[:],
                    gz_sb[:, j, :NX],
                    gz_sb[:, j, NX:NX + Bh],
                    start=(j == 0),
                    stop=(j == NT - 1),
                )
            mm.then_inc(pe_sem, 1)
            for j in range(NT):
                mm = tensor.matmul(
                    acc_b[:],
                    gz_sb[:, j, :NX],
                    gz_sb[:, j, NX + Bh:],
                    start=(j == 0),
                    stop=(j == NT - 1),
                )
            mm.then_inc(pe_sem, 1)

        @block.scalar
        def _(scalar):
            scalar.activation(
                out_sb[:, :Bh], acc_a[:],
                mybir.ActivationFunctionType.Copy,
            )._wait_ge(pe_sem, 1).then_inc(cp_sem, 1)

        @block.vector
        def _(vector):
            vector.tensor_copy(
                out_sb[:, Bh:], acc_b[:]
            )._wait_ge(pe_sem, 2).then_inc(cp_sem, 1)

    return nc


def kernel(**inputs):
    global last_run, last_sim_ns
    Yp = np.asarray(inputs["Yp"], dtype=np.float32)
    Up = np.asarray(inputs["Up"], dtype=np.float32)
    Dp = np.asarray(inputs["Dp"], dtype=np.float32)
    T, B, NY = Yp.shape
    NU = Up.shape[2]
    ND = Dp.shape[2]
    NX = np.asarray(inputs["Wfx"]).shape[0]
    NZ = NU + ND + NY
    W = NX + B

    G, gsum, Kw = _precompute_G(
        T,
        *(np.asarray(inputs[k]) for k in (
            "Wfx", "bfx", "Wfu", "bfu", "Wfd", "bfd", "Wfy", "bfy",
            "Q", "R", "P0", "L0", "x0")),
    )
    t0 = T - Kw
    Rall = Kw * NZ
    Gf = G.reshape(Rall, NX)

    # Z rows aligned with G rows: per step t, rows = [u (NU); d (ND); ym (NY)]
    Z = np.empty((Kw, NZ, B), dtype=np.float32)
    Z[:, :NU] = Up[t0:].transpose(0, 2, 1)
    Z[:, NU:NU + ND] = Dp[t0:].transpose(0, 2, 1)
    Z[:, NU + ND:] = Yp[t0:].transpose(0, 2, 1)
    Zf = Z.reshape(Rall, B)

    R = N_CORES * ROWS_PER_CORE
    if R < Rall:
        rownorm = np.linalg.norm(Gf, axis=1)
        keep = np.argsort(rownorm)[::-1][:R]
        keep.sort()
    else:
        keep = np.arange(Rall)

    GZ = np.zeros((R, W), dtype=ml_dtypes.bfloat16)
    GZ[:len(keep), :NX] = Gf[keep].astype(ml_dtypes.bfloat16)
    GZ[:len(keep), NX:] = Zf[keep].astype(ml_dtypes.bfloat16)

    Rc = ROWS_PER_CORE
    P = Rc // N_TILES
    key = (P, N_TILES, B, NX)
    if key not in _built_cache:
        _built_cache[key] = _build_bass(P, N_TILES, B, NX)
    in_maps = [
        {"gz": np.ascontiguousarray(GZ[c * Rc:(c + 1) * Rc])}
        for c in range(N_CORES)
    ]
    try:
        res = run_bass_kernel_spmd(_built_cache[key], in_maps,
                                   core_ids=list(range(N_CORES)))
    except Exception:
        # the axon-tunneled device intermittently reports
        # NRT_EXEC_UNIT_UNRECOVERABLE; one retry salvages the call when the
        # failure is per-execute rather than process-fatal
        res = run_bass_kernel_spmd(_built_cache[key], in_maps,
                                   core_ids=list(range(N_CORES)))
    last_run = res

    acc = np.zeros((NX, B), dtype=np.float64)
    for c in range(N_CORES):
        acc += res.results[c]["out"].astype(np.float64)

    if last_sim_ns is None:
        try:
            from concourse.timeline_sim import TimelineSim
            last_sim_ns = TimelineSim(_built_cache[key], no_exec=True).simulate()
        except Exception:
            last_sim_ns = None

    x = acc.T + gsum[None, :]
    return x.astype(np.float32)
